# revision 1
# baseline (speedup 1.0000x reference)
"""DCNv2 (modulated deformable conv) Trainium2 Bass kernel.

Sharding: 8 cores = (batch b in 2) x (H-quarter q in 4); each core computes
out[b, :, 16q:16q+16, :] (256 out-channels x 1024 pixels).

Algorithm per core:
  1. offset/mask 3x3 conv on an 18-row slab -> om [27, 1024] (fp32 PE matmuls;
     host permutes weight rows so om = [dy(9) | dx(9) | m(9)]).
  2. geometry on DVE/ACT: ys/xs, floor, fractions, mask=sigmoid, lambda planes
     {mask, mask*ly, mask*lx, mask*ly*lx}, gather indices into a padded grid.
  3. single dma_gather (SWDGE, transposed, bf16) per tap from a host-built
     table[(H+12)^2 rows, 1024] whose row p = [T1 | Ty | Tx | Txy] (value +
     finite differences, zero-padded) -> channels-on-partitions.
  4. bilinear via the lambda identity: val = l1*T1 + ly*Ty + lx*Tx + lxy*Txy
     (DVE bf16, lambda planes broadcast to 128 partitions via K=1 matmuls).
  5. main conv: psum[o, p] += sum_{tap, c-half} wdcnT^T @ val  (bf16 PE).
"""
import os
import sys
import numpy as np

sys.path.insert(0, "/opt/trn_rl_repo")

from contextlib import ExitStack
import concourse.bass as bass
import concourse.bacc as bacc
import concourse.tile as tile
import concourse.mybir as mybir
from concourse.bass_utils import run_bass_kernel_spmd


AF = mybir.ActivationFunctionType
ALU = mybir.AluOpType
DT = mybir.dt

B, C, H, W, K2 = 2, 256, 64, 64, 9
STAGE = int(os.environ.get("DCN_STAGE", "9"))
PADG = 6
G = H + 2 * PADG            # 76 padded grid side
NROWS = G * G               # 5776 table rows
HQ = 16                     # output rows per core
T = HQ * W                  # 1024 pixels per core
N_CORES = 8
MAGIC = 12582912.0          # 1.5 * 2^23 round-to-int magic

_cache = {}


def _build_module():
    nc = bacc.Bacc("TRN2", debug=False, num_devices=N_CORES,
                   dynamic_dma_scratch_size=32768)

    # ---- dram tensors -------------------------------------------------------
    slab = nc.dram_tensor("slab", [C, 18, 66], DT.float32, kind="ExternalInput")
    womT = nc.dram_tensor("womT", [18, 128, 27], DT.float32, kind="ExternalInput")
    bom = nc.dram_tensor("bom", [27, 1], DT.float32, kind="ExternalInput")
    wdT = nc.dram_tensor("wdT", [18, 128, 256], DT.float16, kind="ExternalInput")
    bdcn = nc.dram_tensor("bdcn", [128, 2], DT.float32, kind="ExternalInput")
    baseyx = nc.dram_tensor("baseyx", [K2, 2, T], DT.float32, kind="ExternalInput")
    table = nc.dram_tensor("table", [NROWS, 1024], DT.float16, kind="ExternalInput")
    sel = nc.dram_tensor("sel", [K2, K2 * 128], DT.float32, kind="ExternalInput")
    out = nc.dram_tensor("out", [C, T], DT.float32, kind="ExternalOutput")
    ibounce = nc.dram_tensor("ibounce", [K2, T], DT.int16)
    obounce = nc.dram_tensor("obounce", [27, T], DT.float32)

    with tile.TileContext(nc) as tc, ExitStack() as ctx:
        consts = ctx.enter_context(tc.tile_pool(name="consts", bufs=1))
        geom = ctx.enter_context(tc.tile_pool(name="geom", bufs=1))
        ompsum = ctx.enter_context(tc.tile_pool(name="ompsum", bufs=2, space="PSUM"))
        ppsum = ctx.enter_context(tc.tile_pool(name="ppsum", bufs=1, space="PSUM"))
        opsum = ctx.enter_context(tc.tile_pool(name="opsum", bufs=1, space="PSUM"))
        planes = ctx.enter_context(tc.tile_pool(name="planes", bufs=2))
        gbuf = ctx.enter_context(tc.tile_pool(name="gbuf", bufs=2))
        cbuf = ctx.enter_context(tc.tile_pool(name="cbuf", bufs=2))

        # ---- load constants -------------------------------------------------
        t_slab = []
        for ch in range(2):
            s = consts.tile([128, 18, 66], DT.float32, name=f"slab{ch}")
            nc.sync.dma_start(s[:], slab.ap()[128 * ch:128 * (ch + 1)])
            t_slab.append(s)
        t_womT = consts.tile([128, 18, 27], DT.float32)
        nc.sync.dma_start(t_womT[:], womT.ap().transpose([1, 0, 2]))
        t_wdT = consts.tile([128, 18, 256], DT.float16)
        nc.sync.dma_start(t_wdT[:], wdT.ap().transpose([1, 0, 2]))
        t_bom = consts.tile([27, 1], DT.float32)
        nc.sync.dma_start(t_bom[:], bom.ap())
        t_bdcn = consts.tile([128, 2], DT.float32)
        nc.sync.dma_start(t_bdcn[:], bdcn.ap())
        t_baseyx = consts.tile([K2, 2, T], DT.float32)
        nc.sync.dma_start(t_baseyx[:], baseyx.ap())
        t_sel = consts.tile([K2, K2 * 128], DT.float32)
        nc.sync.dma_start(t_sel[:], sel.ap())

        # ---- offset conv: om27 rows = [dy(9) | dx(9) | m(9)] ---------------
        t_om27 = geom.tile([27, T], DT.float32, name="om27")
        for nh in range(2):
            ps = ompsum.tile([27, 512], DT.float32)
            i = 0
            for k in range(K2):
                ky, kx = k // 3, k % 3
                for ch in range(2):
                    rhs = t_slab[ch][:, 8 * nh + ky: 8 * nh + ky + 8, kx:kx + 64]
                    nc.tensor.matmul(ps[:], t_womT[:, 2 * k + ch], rhs,
                                     start=(i == 0), stop=(i == 17))
                    i += 1
            nc.scalar.activation(t_om27[:, 512 * nh:512 * (nh + 1)], ps[:],
                                 AF.Identity, bias=t_bom[:])

        # rearrange [3*9, T] -> [9, 3, T] via DRAM bounce (partition-crossing)
        nc.sync.dma_start(obounce.ap(), t_om27[:])
        t_omr = geom.tile([K2, 3, T], DT.float32, name="omr")
        nc.sync.dma_start(t_omr[:],
                          bass.AP(obounce, 0, [[T, K2], [K2 * T, 3], [1, T]]))

        # ---- geometry (all tiles at base partition 0) -----------------------
        def g9(name, inner=1):
            return geom.tile([K2, inner, T], DT.float32, name=name)

        t_ysxs = g9("ysxs", 2)
        nc.vector.tensor_add(t_ysxs[:], t_omr[:, 0:2], t_baseyx[:])
        t_mask = g9("mask")
        nc.scalar.activation(t_mask[:], t_omr[:, 2:3], AF.Sigmoid)

        # floor + frac on [9, 2, T] wholesale
        t_r = g9("om27", 2)
        nc.vector.tensor_scalar(t_r[:], t_ysxs[:], MAGIC, -MAGIC, ALU.add, ALU.add)
        t_gt = g9("gttmp", 2)
        nc.vector.tensor_tensor(t_gt[:], t_r[:], t_ysxs[:], ALU.is_gt)
        t_fl = g9("fl", 2)     # [9, 0, T]=y0  [9, 1, T]=x0
        nc.vector.tensor_sub(t_fl[:], t_r[:], t_gt[:])
        t_fr = g9("fr", 2)     # ly | lx
        nc.vector.tensor_sub(t_fr[:], t_ysxs[:], t_fl[:])

        t_lamy = g9("lamy"); t_lamx = g9("lamx"); t_lamxy = g9("lamxy")
        nc.vector.tensor_mul(t_lamy[:], t_mask[:], t_fr[:, 0:1])
        nc.vector.tensor_mul(t_lamx[:], t_mask[:], t_fr[:, 1:2])
        nc.vector.tensor_mul(t_lamxy[:], t_lamy[:], t_fr[:, 1:2])

        # gather index: idx = clip(y0+PADG, 0, G-1)*G + clip(x0+PADG, 0, G-1)
        t_yp = g9("ryp")
        nc.vector.tensor_scalar(t_yp[:], t_fl[:, 0:1], float(PADG), 0.0,
                                ALU.add, ALU.max)
        t_ypg = g9("ysxs")
        nc.vector.tensor_scalar(t_ypg[:], t_yp[:], float(G - 1), float(G),
                                ALU.min, ALU.mult)
        t_xp = g9("ryp")
        nc.vector.tensor_scalar(t_xp[:], t_fl[:, 1:2], float(PADG), 0.0,
                                ALU.add, ALU.max)
        t_xpc = g9("gttmp")
        nc.vector.tensor_scalar(t_xpc[:], t_xp[:], float(G - 1), None, ALU.min)
        t_idx = g9("fr")
        nc.vector.tensor_add(t_idx[:], t_ypg[:], t_xpc[:])

        # ---- index export: cast + wrap + replicate -------------------------
        t_widx = consts.tile([128, 576], DT.int16)
        if STAGE >= 3:
            nc.gpsimd.dma_start(ibounce.ap(), t_idx[:, 0])  # fp32 -> int16
            src_v = bass.AP(ibounce, 0, [[1, 16], [1024, K2], [16, 64]])
            for gidx in range(8):
                nc.sync.dma_start(
                    t_widx[16 * gidx:16 * (gidx + 1), :]
                    .rearrange("p (k s) -> p k s", k=K2), src_v)
        else:
            nc.any.memset(t_widx[:], 0)

        # ---- per-tap: planes, gather, combine, matmul ----------------------
        t_osum = [opsum.tile([128, T], DT.float32, name=f"osum{oh}")
                  for oh in range(2)]
        lam_srcs = [t_mask, t_lamy, t_lamx, t_lamxy]
        # note: lam tiles are [9, 1, T]; squeeze for matmul rhs
        for k in range(K2 if STAGE >= 4 else 0):
            pls = []
            for pi in range(4):
                pp = ppsum.tile([128, T], DT.float32, name="planepsum")
                for nh in range(2):
                    nc.tensor.matmul(pp[:, 512 * nh:512 * (nh + 1)],
                                     t_sel[:, 128 * k:128 * (k + 1)],
                                     lam_srcs[pi][:, 0, 512 * nh:512 * (nh + 1)],
                                     start=True, stop=True)
                pl = planes.tile([128, T], DT.float16, name=f"plane{pi}")
                nc.scalar.activation(pl[:], pp[:], AF.Copy)
                pls.append(pl)

            gcs = []
            for (co, cn) in ((0, 384), (384, 384), (768, 256)):
                gc = gbuf.tile([128, 8, cn], DT.float16, name="gath")
                if STAGE < 5:
                    nc.any.memset(gc[:], 0.25)
                else:
                    nc.gpsimd.dma_gather(
                        out_ap=gc[:],
                        in_ap=table.ap(),
                        idxs_ap=t_widx[:, 64 * k + co // 16:
                                       64 * k + (co + cn) // 16],
                        num_idxs=cn,
                        num_idxs_reg=cn,
                        elem_size=1024,
                        transpose=True,
                    )
                gcs.append((gc, co, cn))

            def bc(pl):  # [128, T] -> [128, 2, T] broadcast along groups
                return pl[:].unsqueeze(1).broadcast_to([128, 2, T])

            val = cbuf.tile([128, 2, T], DT.float16, name="val")
            for gc, co, cn in gcs:
                gv = gc[:]

                def bcc(pl):
                    return (pl[:, co:co + cn].unsqueeze(1)
                            .broadcast_to([128, 2, cn]))

                ca1 = cbuf.tile([128, 2, cn], DT.float16, name="ca")
                nc.vector.tensor_mul(ca1[:], gv[:, 0:2], bcc(pls[0]))
                cb1 = cbuf.tile([128, 2, cn], DT.float16, name="cb")
                nc.vector.tensor_mul(cb1[:], gv[:, 2:4], bcc(pls[1]))
                ca2 = cbuf.tile([128, 2, cn], DT.float16, name="ca")
                nc.vector.tensor_add(ca2[:], ca1[:], cb1[:])
                cb2 = cbuf.tile([128, 2, cn], DT.float16, name="cb")
                nc.vector.tensor_mul(cb2[:], gv[:, 4:6], bcc(pls[2]))
                ca3 = cbuf.tile([128, 2, cn], DT.float16, name="ca")
                nc.vector.tensor_mul(ca3[:], gv[:, 6:8], bcc(pls[3]))
                cb3 = cbuf.tile([128, 2, cn], DT.float16, name="cb")
                nc.vector.tensor_add(cb3[:], cb2[:], ca3[:])
                nc.vector.tensor_add(val[:, :, co:co + cn], ca2[:], cb3[:])

            for ch in range(2):
                lhsT = t_wdT[:, 2 * k + ch]            # [128, 256]
                for oh in range(2):
                    for nh in range(2):
                        nc.tensor.matmul(
                            t_osum[oh][:, 512 * nh:512 * (nh + 1)],
                            lhsT[:, 128 * oh:128 * (oh + 1)],
                            val[:, ch, 512 * nh:512 * (nh + 1)],
                            start=(k == 0 and ch == 0),
                            stop=(k == K2 - 1 and ch == 1),
                        )

        # ---- output ---------------------------------------------------------
        for oh in range(2):
            osb = cbuf.tile([128, T], DT.float32, name="osb")
            if STAGE >= 4:
                nc.scalar.activation(osb[:], t_osum[oh][:], AF.Identity,
                                     bias=t_bdcn[:, oh:oh + 1])
            elif STAGE >= 2:
                nc.any.memset(osb[:], 0.0)
                nc.vector.tensor_copy(osb[0:9, :], t_lamy[:, 0])
            else:
                nc.any.memset(osb[:], 0.0)
                nc.vector.tensor_copy(osb[0:27, :], t_om27[:])
            nc.sync.dma_start(out.ap()[128 * oh:128 * (oh + 1)], osb[:])

    nc.compile()
    return nc


def _host_prep(x, offset_feat, w_offset_mask, b_offset_mask, w_dcn, b_dcn):
    perm = list(range(0, 18, 2)) + list(range(1, 18, 2)) + list(range(18, 27))
    w_om_p = w_offset_mask[perm].astype(np.float32)      # [27, 256, 3, 3]
    b_om_p = b_offset_mask[perm].astype(np.float32)

    # womT[k*2+ch] = [128 c, 27] for tap k, channel half ch
    womT = np.zeros((18, 128, 27), np.float32)
    for k in range(9):
        ky, kx = k // 3, k % 3
        wt = w_om_p[:, :, ky, kx]                        # [27, 256]
        for ch in range(2):
            womT[2 * k + ch] = wt[:, 128 * ch:128 * (ch + 1)].T

    # wdT[k*2+ch] = [128 c, 256 o]
    wd = w_dcn.reshape(C, C, 9).astype(np.float32)       # [o, c, k]
    wdT = np.zeros((18, 128, 256), np.float32)
    for k in range(9):
        for ch in range(2):
            wdT[2 * k + ch] = wd[:, 128 * ch:128 * (ch + 1), k].T
    wdT = wdT.astype(np.float16)

    bdcn = b_dcn.astype(np.float32).reshape(2, 128).T.copy()  # [128, 2]

    # tables per batch
    tables = []
    for b in range(B):
        xp = np.zeros((C, G + 1, G + 1), np.float32)
        xp[:, PADG:PADG + H, PADG:PADG + W] = x[b]
        T1 = xp[:, :G, :G]
        Ty = xp[:, 1:, :G] - T1
        Tx = xp[:, :G, 1:] - T1
        Txy = xp[:, 1:, 1:] - xp[:, 1:, :G] - xp[:, :G, 1:] + T1
        tab = np.concatenate(
            [t.reshape(C, NROWS).T for t in (T1, Ty, Tx, Txy)], axis=1)
        tables.append(np.ascontiguousarray(tab.astype(np.float16)))

    # base grids per h-quarter
    ky = np.repeat(np.arange(3), 3)[:, None].astype(np.float32)
    kx = np.tile(np.arange(3), 3)[:, None].astype(np.float32)
    t_ho = (np.arange(T) // W)[None, :].astype(np.float32)
    t_wo = (np.arange(T) % W)[None, :].astype(np.float32)
    in_maps = []
    selv = np.zeros((K2, K2 * 128), np.float32)
    for k in range(K2):
        selv[k, 128 * k:128 * (k + 1)] = 1.0
    for b in range(B):
        for q in range(4):
            ho0 = q * HQ
            slab = np.zeros((C, 18, 66), np.float32)
            r0 = ho0 - 1
            rr0, rr1 = max(r0, 0), min(ho0 + 17, H)
            slab[:, rr0 - r0:rr1 - r0, 1:65] = offset_feat[b][:, rr0:rr1, :]
            in_maps.append({
                "slab": slab,
                "womT": womT,
                "bom": b_om_p.reshape(27, 1),
                "wdT": wdT,
                "bdcn": bdcn,
                "baseyx": np.stack([(ho0 + t_ho + ky - 1),
                                    np.broadcast_to(t_wo + kx - 1, (K2, T))],
                                   axis=1).astype(np.float32),
                "table": tables[b],
                "sel": selv,
            })
    return in_maps


def _get_module():
    if "nc" not in _cache:
        _cache["nc"] = _build_module()
    return _cache["nc"]


def kernel(x, offset_feat, w_offset_mask, b_offset_mask, w_dcn, b_dcn,
           **run_kwargs):
    x = np.asarray(x); offset_feat = np.asarray(offset_feat)
    w_offset_mask = np.asarray(w_offset_mask)
    b_offset_mask = np.asarray(b_offset_mask)
    w_dcn = np.asarray(w_dcn); b_dcn = np.asarray(b_dcn)

    nc = _get_module()
    in_maps = _host_prep(x, offset_feat, w_offset_mask, b_offset_mask,
                         w_dcn, b_dcn)
    res = run_bass_kernel_spmd(nc, in_maps, list(range(N_CORES)), **run_kwargs)
    y = np.zeros((B, C, H, W), np.float32)
    for ci, (b, q) in enumerate([(b, q) for b in range(B) for q in range(4)]):
        y[b, :, q * HQ:(q + 1) * HQ, :] = \
            res.results[ci]["out"].reshape(C, HQ, W)
    kernel.last_results = res
    return y

